# revision 17
# baseline (speedup 1.0000x reference)
import sys

for _p in ("/opt/trn_rl_repo",):
    if _p not in sys.path:
        sys.path.insert(0, _p)

import numpy as np

import bass_rust
import concourse.bass as bass
import concourse.mybir as mybir
import concourse.tile as tile

F32 = mybir.dt.float32
FP16 = mybir.dt.float16
BF16 = mybir.dt.bfloat16

P = 128
HWIDTH = 64
HW = 4096
WROWS = 34
KW = WROWS * HWIDTH
NKC = KW // P
SEG = 16
NSEG = HW // SEG
TOPK = 3

_MAXW = 1
_orig_lower = tile.TileContext._lower_ordered_insts


def _split_waits(tc, ordered):
    nc = tc.nc
    for _bb, insts in ordered.items():
        out = []
        for inst in insts:
            si = inst.sync_info
            if si is not None and len(si.on_wait) > _MAXW:
                waits = list(si.on_wait)
                for w in waits[_MAXW:]:
                    ev = mybir.InstEventSemaphore(
                        name=nc.get_next_instruction_name(), ins=[], outs=[])
                    ev.engine = inst.engine
                    ev.sync_info = bass_rust.SyncInfo(on_wait=[w], on_update=[])
                    out.append(ev)
                inst.sync_info = bass_rust.SyncInfo(
                    on_wait=waits[:_MAXW], on_update=list(si.on_update))
            out.append(inst)
        insts[:] = out


def _lower_patched(self, ordered):
    _split_waits(self, ordered)
    return _orig_lower(self, ordered)


def _drain_and_barrier_split(self, tick_clock, wait_clock):
    nc = self.nc
    probe = mybir.InstNoOp(name=nc.get_next_instruction_name(), ins=[], outs=[])
    probe.engine = mybir.EngineType.SP
    wait_clock.add_sem_waits(
        probe, bass_rust.ScopedClock({None: tick_clock.global_clock}))
    si = probe.sync_info
    waits = list(si.on_wait) if si is not None else []
    assert self.sems is not None
    handles = self.sems.allocated()
    by_name = {}
    for h in handles.values():
        nm = getattr(h, "name", None)
        if nm is not None:
            by_name[nm] = h
    for w in waits:
        h = handles.get(w.ant_name) or by_name.get(w.ant_name)
        assert h is not None, f"no sem handle for {w.ant_name}"
        nc.sync.wait_ge(h, w.wait_value)
    nc.sync.drain()
    nc.all_engine_barrier()
    popped = nc._tile_sem_poison_stack.pop()
    assert popped is self._sem_poison
    nc.clear_and_free_semaphores(list(self.sems.allocated().values()))
    nc.all_engine_barrier()


tile.TileContext._lower_ordered_insts = _lower_patched
tile.TileContext._drain_and_barrier = _drain_and_barrier_split



def build_program():
    nc = bass.Bass()
    AF = mybir.ActivationFunctionType
    OP = mybir.AluOpType
    X = mybir.AxisListType.X

    din = {}
    for name, shape in [
        ("xw", [P, WROWS, HWIDTH]),
        ("xf", [P, HW]),
        ("xb", [P, HW]),
        ("wq4", [P, P]),
        ("wk4", [P, P]),
        ("wvt", [P, P]),
        ("wf", [P, 18, P]),
        ("ident", [P, P]),
        ("iota_d", [P, NSEG]),
    ]:
        din[name] = nc.dram_tensor(name, shape, F32, kind="ExternalInput")
    din["inv_mask"] = nc.dram_tensor("inv_mask", [P, NKC], mybir.dt.uint32,
                                     kind="ExternalInput")
    out_d = nc.dram_tensor("out", [P, 32, HWIDTH], F32, kind="ExternalOutput")
    v_t = nc.dram_tensor("v_t", [HW + 1, P], F32)
    m_dram = nc.dram_tensor("m_dram", [KW], F32)
    qt_blk = nc.dram_tensor("qt_blk", [NSEG, SEG * 16], F32)

    from contextlib import ExitStack
    with tile.TileContext(nc) as tc, ExitStack() as _stk:
        cst = _stk.enter_context(tc.tile_pool(name="cst", bufs=1))
        x_pad = cst.tile([P, WROWS, HWIDTH + 2], F32)
        x_bf = cst.tile([P, WROWS, HWIDTH + 2], BF16)
        sel_bf = cst.tile([P, WROWS, HWIDTH + 2], BF16)
        xf_sb = cst.tile([P, HW], F32)
        q4x = cst.tile([P, HW], F32)
        k4x = cst.tile([P, KW], F32)
        q4r = cst.tile([P, HW], FP16)
        k4r = cst.tile([P, KW], FP16)
        wq4_sb = cst.tile([P, P], F32)
        wk4_sb = cst.tile([P, P], F32)
        wvt_sb = cst.tile([P, P], F32)
        wvt_bf = cst.tile([P, P], BF16)
        wf_sb = cst.tile([P, 18, P], F32)
        wf_bf = cst.tile([P, 18, P], BF16)
        ident_sb = cst.tile([P, P], F32)
        iota_sb = cst.tile([P, NSEG], F32)
        inv_sb = cst.tile([P, NKC], mybir.dt.uint32)
        m_all = cst.tile([P, NKC], F32)
        kt_all = cst.tile([P, NKC, 16], F32)
        c4096 = cst.tile([P, 1], F32)
        dots_all = cst.tile([P, NKC, TOPK * SEG], F32)
        seg3_all = cst.tile([P, NKC, 3], F32)
        zrow = cst.tile([1, P], F32)
        xb_bf = cst.tile([P, HW], BF16)
        selT_all = cst.tile([P, KW], F32)

        for t, name in [(xf_sb, "xf"), (wq4_sb, "wq4"), (wk4_sb, "wk4"),
                        (wvt_sb, "wvt"), (wf_sb, "wf"), (ident_sb, "ident"),
                        (iota_sb, "iota_d"), (inv_sb, "inv_mask")]:
            nc.sync.dma_start(out=t[:], in_=din[name][:])
        nc.gpsimd.memset(x_pad[:], 0.0)
        nc.gpsimd.memset(sel_bf[:], 0.0)
        nc.sync.dma_start(out=x_pad[:, :, 1:65], in_=din["xw"][:])
        nc.vector.memset(c4096[:], 4096.0)
        nc.vector.memset(zrow[:], 0.0)
        nc.sync.dma_start(out=v_t[HW:HW + 1, :], in_=zrow[:])

        xwin = x_pad[:, :, 1:65]

        with tc.tile_pool(name="work", bufs=2) as phb, \
             tc.tile_pool(name="work4", bufs=2) as phs4, \
             tc.tile_pool(name="ps_a", bufs=2, space="PSUM") as psa, \
             tc.tile_pool(name="ps_s", bufs=3, space="PSUM") as pss:
            xb_sb = phb.tile([P, HW], F32, tag="xb")
            nc.sync.dma_start(out=xb_sb[:], in_=din["xb"][:])

            for g in range(8):
                qp = psa.tile([P, 512], F32, tag="ps512")
                nc.tensor.matmul(out=qp[:], lhsT=wq4_sb[:],
                                 rhs=xf_sb[:, 512 * g:512 * (g + 1)],
                                 start=True, stop=True)
                nc.scalar.activation(out=q4x[:, 512 * g:512 * (g + 1)],
                                     in_=qp[:], func=AF.Copy)
            for g in range(4):
                kp = psa.tile([P, 512], F32, tag="ps512")
                nc.tensor.matmul(out=kp[:], lhsT=wk4_sb[:],
                                 rhs=xwin[:, 8 * g:8 * (g + 1), :],
                                 start=True, stop=True)
                nc.scalar.activation(out=k4x[:, 512 * g:512 * (g + 1)],
                                     in_=kp[:], func=AF.Copy)
            kp = psa.tile([P, 512], F32, tag="ps512")
            nc.tensor.matmul(out=kp[:, 0:128], lhsT=wk4_sb[:],
                             rhs=xwin[:, 32:34, :], start=True, stop=True)
            nc.scalar.activation(out=k4x[:, 2048:2176], in_=kp[:, 0:128],
                                 func=AF.Copy)
            nc.scalar.activation(out=q4r[0:16, :], in_=q4x[0:16, :],
                                 func=AF.Copy)
            nc.scalar.activation(out=k4r[0:16, :], in_=k4x[0:16, :],
                                 func=AF.Copy)

            nc.vector.tensor_copy(out=xb_bf[:], in_=xb_sb[:])
            nc.vector.tensor_copy(out=wvt_bf[:], in_=wvt_sb[:])
            nc.vector.tensor_copy(out=wf_bf[:], in_=wf_sb[:])
            nc.vector.tensor_copy(out=x_bf[:], in_=x_pad[:])

            def emit_qt(ch):
                qt_ps = psa.tile([P, 512], F32, tag="ps512")
                nc.tensor.transpose(out=qt_ps[:, 0:16],
                                    in_=q4x[0:16, 128 * ch:128 * (ch + 1)],
                                    identity=ident_sb[0:16, 0:16])
                qts = phb.tile([P, 16], F32, tag="qts")
                nc.scalar.activation(out=qts[:], in_=qt_ps[:, 0:16],
                                     func=AF.Copy)
                dst = bass.AP(qt_blk, 2048 * ch, [[256, 8], [16, 16], [1, 16]])
                nc.sync.dma_start(out=dst, in_=qts[:])

            def emit_kt(kc):
                kt_ps = psa.tile([P, 512], F32, tag="ps512")
                nc.tensor.transpose(out=kt_ps[:, 0:16],
                                    in_=k4x[0:16, 128 * kc:128 * (kc + 1)],
                                    identity=ident_sb[0:16, 0:16])
                nc.scalar.activation(out=kt_all[:, kc, :], in_=kt_ps[:, 0:16],
                                     func=AF.Copy)

            def emit_vt(grp):
                vt_ps = psa.tile([P, 512], F32, tag="ps512")
                for j in range(4):
                    ch = 4 * grp + j
                    nc.tensor.matmul(
                        out=vt_ps[:, 128 * j:128 * (j + 1)],
                        lhsT=xb_bf[:, 128 * ch:128 * (ch + 1)],
                        rhs=wvt_bf[:],
                        start=True, stop=True)
                vts = phb.tile([P, 512], F32, tag="vts")
                nc.scalar.activation(out=vts[:], in_=vt_ps[:], func=AF.Copy)
                dst = bass.AP(v_t, 512 * grp * P,
                              [[P, P], [P * P, 4], [1, P]])
                nc.sync.dma_start(out=dst, in_=vts[:])

            interleave = ([('qt', ch) for ch in range(32)]
                          + [('kt', kc) for kc in range(NKC)]
                          + [('vt', g) for g in range(8)])
            emit_fns = {'qt': emit_qt, 'kt': emit_kt, 'vt': emit_vt}

            idx_all = phb.tile([P, NKC], mybir.dt.uint32, tag="idx_all", bufs=1)
            m_stage = phb.tile([1, KW], F32, tag="m_stage", bufs=1)

            def emit_refine(kc, qblk):
                ktc = kt_all[:, kc, :]
                ktb = bass.AP(ktc.tensor, ktc.offset,
                              [ktc.ap[0], [0, TOPK], [0, SEG], ktc.ap[1]])
                t768 = phs4.tile([P, TOPK, SEG, 16], F32, tag="t768")
                nc.vector.tensor_tensor(
                    out=t768[:],
                    in0=ktb,
                    in1=qblk[:].rearrange("p t (w c) -> p t w c", c=16),
                    op=OP.mult)
                nc.vector.tensor_reduce(
                    out=dots_all[:, kc, :], in_=t768[:], axis=X, op=OP.add)

            def emit_idx_math(lo, hi):
                n = hi - lo
                sl = slice(lo, hi)
                nc.vector.tensor_reduce(out=m_all[:, sl],
                                        in_=dots_all[:, sl, :], axis=X,
                                        op=OP.max)
                moff = m_all[:, sl]
                mb3 = bass.AP(moff.tensor, moff.offset,
                              [moff.ap[0], [1, n], [0, TOPK * SEG]])
                ge_all = phb.tile([P, 9, TOPK * SEG], F32, tag="ge_all")
                nc.vector.tensor_tensor(out=ge_all[:, 0:n, :],
                                        in0=dots_all[:, sl, :], in1=mb3,
                                        op=OP.is_ge)
                iotb = bass.AP(iota_sb.tensor,
                               iota_sb.offset + (NSEG - TOPK * SEG),
                               [iota_sb.ap[0], [0, n], [1, TOPK * SEG]])
                nc.vector.tensor_tensor(out=ge_all[:, 0:n, :],
                                        in0=ge_all[:, 0:n, :], in1=iotb,
                                        op=OP.mult)
                rw = phb.tile([P, 9], F32, tag="rw_all")
                nc.vector.tensor_reduce(out=rw[:, 0:n],
                                        in_=ge_all[:, 0:n, :], axis=X,
                                        op=OP.max)
                slot = phb.tile([P, 9], F32, tag="slot17")
                nc.vector.tensor_scalar(out=slot[:, 0:n], in0=rw[:, 0:n],
                                        scalar1=-1.0,
                                        scalar2=float(TOPK * SEG),
                                        op0=OP.mult, op1=OP.add)
                t1 = phb.tile([P, 9], F32, tag="t1b")
                nc.vector.tensor_scalar(out=t1[:, 0:n], in0=slot[:, 0:n],
                                        scalar1=16.0, scalar2=0.0,
                                        op0=OP.is_ge, op1=OP.bypass)
                t2 = phb.tile([P, 9], F32, tag="t2b")
                nc.vector.tensor_scalar(out=t2[:, 0:n], in0=slot[:, 0:n],
                                        scalar1=32.0, scalar2=0.0,
                                        op0=OP.is_ge, op1=OP.bypass)
                t12 = phb.tile([P, 9], F32, tag="t12b")
                nc.vector.tensor_add(t12[:, 0:n], t1[:, 0:n], t2[:, 0:n])
                wof = phb.tile([P, 9], F32, tag="wofb")
                nc.vector.scalar_tensor_tensor(
                    out=wof[:, 0:n], in0=t12[:, 0:n], scalar=-16.0,
                    in1=slot[:, 0:n], op0=OP.mult, op1=OP.add)
                s0 = seg3_all[:, sl, 0]
                s1 = seg3_all[:, sl, 1]
                s2 = seg3_all[:, sl, 2]
                d10 = phb.tile([P, 9], F32, tag="d10b")
                nc.vector.tensor_sub(d10[:, 0:n], s1, s0)
                d21 = phb.tile([P, 9], F32, tag="d21b")
                nc.vector.tensor_sub(d21[:, 0:n], s2, s1)
                seg = phb.tile([P, 9], F32, tag="segb")
                nc.vector.tensor_tensor(out=seg[:, 0:n], in0=t1[:, 0:n],
                                        in1=d10[:, 0:n], op=OP.mult)
                nc.vector.tensor_add(seg[:, 0:n], seg[:, 0:n], s0)
                nc.vector.tensor_tensor(out=d21[:, 0:n], in0=t2[:, 0:n],
                                        in1=d21[:, 0:n], op=OP.mult)
                nc.vector.tensor_add(seg[:, 0:n], seg[:, 0:n], d21[:, 0:n])
                qf = phb.tile([P, 9], F32, tag="qfb")
                nc.vector.scalar_tensor_tensor(
                    out=qf[:, 0:n], in0=seg[:, 0:n], scalar=16.0,
                    in1=wof[:, 0:n], op0=OP.mult, op1=OP.add)
                nc.vector.tensor_scalar_max(qf[:, 0:n], qf[:, 0:n], 0.0)
                nc.vector.tensor_scalar_min(qf[:, 0:n], qf[:, 0:n],
                                            float(HW - 1))
                c4096b = bass.AP(c4096.tensor, c4096.offset,
                                 [c4096.ap[0], [0, n]])
                nc.vector.copy_predicated(qf[:, 0:n], inv_sb[:, sl], c4096b)
                nc.vector.tensor_copy(out=idx_all[:, sl], in_=qf[:, 0:n])

            def emit_vgather_sel(lo, hi):
                for kc in range(lo, hi):
                    idx_col = idx_all[:, kc:kc + 1]
                    nc.gpsimd.indirect_dma_start(
                        out=selT_all[:, 128 * kc:128 * (kc + 1)],
                        out_offset=None, in_=v_t[:, :],
                        in_offset=bass.IndirectOffsetOnAxis(ap=idx_col, axis=0))
                    sel_ps = psa.tile([P, 512], F32, tag="ps512")
                    nc.tensor.transpose(
                        out=sel_ps[:, 0:128],
                        in_=selT_all[:, 128 * kc:128 * (kc + 1)],
                        identity=ident_sb[:])
                    nc.scalar.activation(
                        out=sel_bf[:, 2 * kc:2 * kc + 2, 1:65],
                        in_=sel_ps[:, 0:128], func=AF.Copy)
                    nc.sync.dma_start(
                        out=m_stage[0:1, 128 * kc:128 * (kc + 1)],
                        in_=m_all[:, kc:kc + 1])

            pend = None
            for kc in range(NKC):
                bm16 = phb.tile([P, NSEG], F32, tag="bm16")
                for qg in range(4):
                    s_ps = pss.tile([P, 1024], F32, tag="s_ps")
                    for j in range(2):
                        nc.tensor.matmul(
                            out=s_ps[:, 512 * j:512 * (j + 1)],
                            lhsT=k4r[0:16, 128 * kc:128 * (kc + 1)],
                            rhs=q4r[0:16, 1024 * qg + 512 * j:
                                    1024 * qg + 512 * (j + 1)],
                            start=True, stop=True)
                    seg_view = s_ps[:].rearrange("p (s w) -> p s w", w=SEG)
                    nc.vector.tensor_reduce(
                        out=bm16[:, 64 * qg:64 * (qg + 1)], in_=seg_view,
                        axis=X, op=OP.max)
                for _ in range(16 if kc < 2 else 7):
                    if interleave:
                        nm, arg = interleave.pop(0)
                        emit_fns[nm](arg)
                top8 = phs4.tile([P, 8], F32, tag="top8")
                idx8 = phs4.tile([P, 8], mybir.dt.uint32, tag="idx8")
                nc.vector.max(out=top8[:], in_=bm16[:])
                nc.vector.max_index(out=idx8[:], in_max=top8[:], in_values=bm16[:])
                nc.vector.tensor_copy(out=seg3_all[:, kc, :], in_=idx8[:, 0:3])
                if kc == 0:
                    idx8_kc0 = phb.tile([P, 8], mybir.dt.uint32,
                                        tag="idx8_kc0", bufs=1)
                    nc.vector.tensor_copy(out=idx8_kc0[:], in_=idx8[:])
                    continue
                qblk = phs4.tile([P, TOPK, 256], F32, tag="qblk")
                for tt in range(TOPK):
                    nc.gpsimd.indirect_dma_start(
                        out=qblk[:, tt, :], out_offset=None, in_=qt_blk[:, :],
                        in_offset=bass.IndirectOffsetOnAxis(
                            ap=idx8[:, tt:tt + 1], axis=0))
                if pend is not None:
                    emit_refine(*pend)
                    if pend[0] == 9:
                        emit_idx_math(1, 10)
                        emit_vgather_sel(1, 10)
                pend = (kc, qblk)
            emit_refine(*pend)
            qblk0 = phs4.tile([P, TOPK, 256], F32, tag="qblk")
            for tt in range(TOPK):
                nc.gpsimd.indirect_dma_start(
                    out=qblk0[:, tt, :], out_offset=None, in_=qt_blk[:, :],
                    in_offset=bass.IndirectOffsetOnAxis(
                        ap=idx8_kc0[:, tt:tt + 1], axis=0))
            emit_refine(0, qblk0)
            emit_idx_math(10, NKC)
            emit_vgather_sel(10, NKC)
            emit_idx_math(0, 1)
            emit_vgather_sel(0, 1)

            nc.sync.dma_start(out=m_dram[:], in_=m_stage[:])
            mb_sb = phb.tile([P, 2048], F32, tag="mb_sb", bufs=1)
            bcast = bass.AP(m_dram, HWIDTH, [[0, P], [1, 2048]])
            nc.sync.dma_start(out=mb_sb[:], in_=bcast)

            for g in range(4):
                cv = psa.tile([P, 512], F32, tag="ps512")
                t = 0
                for half in range(2):
                    src_pad = x_bf if half == 0 else sel_bf
                    for dy in range(3):
                        for dx in range(3):
                            nc.tensor.matmul(
                                out=cv[:],
                                lhsT=wf_bf[:, t, :],
                                rhs=src_pad[:, 8 * g + dy:8 * g + 8 + dy,
                                            dx:dx + HWIDTH],
                                start=(t == 0), stop=(t == 17))
                            t += 1
                ob = phb.tile([P, 512], F32, tag="ob")
                nc.vector.tensor_tensor(
                    out=ob[:], in0=cv[:], in1=mb_sb[:, 512 * g:512 * (g + 1)],
                    op=OP.mult)
                nc.vector.tensor_tensor(
                    out=ob[:].rearrange("p (a b) -> p a b", b=HWIDTH),
                    in0=ob[:].rearrange("p (a b) -> p a b", b=HWIDTH),
                    in1=x_pad[:, 8 * g + 1:8 * g + 9, 1:65], op=OP.add)
                nc.sync.dma_start(
                    out=out_d[:, 8 * g:8 * (g + 1), :],
                    in_=ob[:].rearrange("p (a b) -> p a b", b=HWIDTH))

    return nc



def _host_inputs(x, x_forward, x_backward, Wq, Wk, Wv, Wf):
    B = x.shape[0]
    wq4 = np.zeros((P, P), np.float32)
    wk4 = np.zeros((P, P), np.float32)
    for i in range(4):
        wq4[:, 32 * i:32 * i + 16] = Wq.T.astype(np.float32)
        wk4[:, 32 * i:32 * i + 16] = Wk.T.astype(np.float32)
    wvt = np.ascontiguousarray(Wv.T.astype(np.float32))
    wf = np.ascontiguousarray(
        Wf.reshape(P, 2, P, 3, 3).transpose(2, 1, 3, 4, 0)
        .reshape(P, 18, P).astype(np.float32))
    ident = np.eye(P, dtype=np.float32)
    iota_d = np.broadcast_to(
        (NSEG - np.arange(NSEG, dtype=np.float32)), (P, NSEG)).copy()

    maps = []
    for d in range(8):
        b, half = d // 2, d % 2
        row0 = half * 32 - 1
        xw = np.zeros((P, WROWS, HWIDTH), np.float32)
        rlo, rhi = max(0, row0), min(64, row0 + WROWS)
        xw[:, rlo - row0:rhi - row0, :] = x[b, :, rlo:rhi, :]
        inv = np.zeros((P, NKC), np.uint32)
        if half == 0:
            inv[0:64, 0] = 1
        else:
            inv[64:128, NKC - 1] = 1
        maps.append({
            "xw": xw,
            "xf": np.ascontiguousarray(
                x_forward[b].reshape(P, HW).astype(np.float32)),
            "xb": np.ascontiguousarray(
                x_backward[b].reshape(P, HW).astype(np.float32)),
            "wq4": wq4, "wk4": wk4, "wvt": wvt, "wf": wf, "ident": ident,
            "iota_d": iota_d, "inv_mask": inv,
        })
    return maps


_CACHE = {}


def _get_program():
    if "nc" not in _CACHE:
        _CACHE["nc"] = build_program()
    return _CACHE["nc"]


def run(inputs, trace=False):
    from concourse.bass_utils import run_bass_kernel_spmd
    nc = _get_program()
    maps = _host_inputs(inputs["x"], inputs["x_forward"], inputs["x_backward"],
                        inputs["Wq"], inputs["Wk"], inputs["Wv"], inputs["Wf"])
    res = run_bass_kernel_spmd(nc, maps, core_ids=list(range(8)), trace=trace)
    B = inputs["x"].shape[0]
    out = np.zeros((B, P, 64, HWIDTH), np.float32)
    for d in range(8):
        b, half = d // 2, d % 2
        out[b, :, 32 * half:32 * (half + 1), :] = res.results[d]["out"]
    return out, res


def kernel(**inputs):
    inputs = {k: np.asarray(v) for k, v in inputs.items()}
    out, _ = run(inputs, trace=False)
    return out
